# revision 31
# baseline (speedup 1.0000x reference)
# Trainium2 Bass kernel for DST_Decoder (v5).
#
# Math (exact):
#   h  = relu(x @ w1 + b1);  p = h @ w2 + b2                  (pointwise MLP)
#   dx_t = p_t - p_{t-1} (p_0 basepoint=0);  m2_t = p_t + p_{t-1} (= 2*m_t)
#   sig increment u_t = vec(m2_t (x) dx_t) @ (W1_sig/2) + dx_t @ W1_s1
#   z = cumsum_t(u);  out = relu(z + bb1) @ W2 + bb2
#
# Layout: transposed land (channels on partitions, time on free axis).
# chunk = L = 1024 so one chunk == one batch element; 4 per core.
#
# Perf notes (vs 149.9us v1 baseline):
#  - all matmuls bf16; 104 matmuls of 512 free (PSUM-bank limit) run at
#    ~490ns each once the PE stays hot
#  - the m-broadcast runs on the DMA fabric, bounced through DRAM in a
#    partition-interleaved pair mapping (i = 4r + q%4, j = q//4) so each
#    broadcast lowers to many descriptors -> spreads over all 16 DMA engines
#    (descriptors round-robin; few big descriptors pile onto 4 engines)
#  - outer-product multiply in place on bf16 SBUF tiles (DVE 2x mode)
#  - scan reads mains PSUM directly; per-batch scan/relu/head keeps the tail
#    short; phase1s all issue before the first mains so the PE never starves

import os
import sys

import numpy as np

for _p in ("/opt/trn_rl_repo",):
    if _p not in sys.path and os.path.isdir(_p):
        sys.path.append(_p)

from concourse import bacc, tile
from concourse import bass_utils
import concourse.mybir as mybir

F32 = mybir.dt.float32
BF16 = mybir.dt.bfloat16

N_CORES = 8
B, L, DIN = 32, 1024, 256
C, HID, DOUT = 32, 64, 128
B_CORE = B // N_CORES                 # 4 batches per core
T = B_CORE * L                        # 4096 time positions per core
KT = (C * C) // 128                   # 8 k-tiles of the outer-product block

TRACE = False
LAST_EXEC_NS = None
LAST_PROFILE = None
LAST_TRACE_PATH = None


def build_nc():
    nb = B_CORE                       # chunks == batches per core
    nc = bacc.Bacc(trn_type="TRN2", target_bir_lowering=False, debug=False)

    xT = nc.dram_tensor("xT", (128, 2, T), BF16, kind="ExternalInput").ap()
    w1d = nc.dram_tensor("w1d", (128, 2, HID), BF16, kind="ExternalInput").ap()
    b1c = nc.dram_tensor("b1c", (HID, 1), F32, kind="ExternalInput").ap()
    w2d = nc.dram_tensor("w2d", (HID, C), BF16, kind="ExternalInput").ap()
    b2c = nc.dram_tensor("b2c", (C, 1), F32, kind="ExternalInput").ap()
    W1md = nc.dram_tensor("W1md", (128, KT, HID), BF16, kind="ExternalInput").ap()
    Vd = nc.dram_tensor("Vd", (C, HID), BF16, kind="ExternalInput").ap()
    bb1s = nc.dram_tensor("bb1s", (128, 1), F32, kind="ExternalInput").ap()
    W2d = nc.dram_tensor("W2d", (2 * HID, DOUT), BF16, kind="ExternalInput").ap()
    bb2c = nc.dram_tensor("bb2c", (DOUT, 1), F32, kind="ExternalInput").ap()
    out = nc.dram_tensor("out", (DOUT, T), F32, kind="ExternalOutput").ap()
    # DRAM bounce buffers for the broadcast-stacked operands
    mTrD = nc.dram_tensor("mTrD", (B_CORE, 4, KT, L), BF16, kind="Internal").ap()
    dxbD = nc.dram_tensor("dxbD", (B_CORE, C, L), BF16, kind="Internal").ap()

    RELU = mybir.ActivationFunctionType.Relu
    ADD = mybir.AluOpType.add
    MUL = mybir.AluOpType.mult
    BYP = mybir.AluOpType.bypass

    with tile.TileContext(nc) as tc:
        with (
            tc.tile_pool(name="consts", bufs=1) as cpool,
            tc.tile_pool(name="xin", bufs=4) as xpool,
            tc.tile_pool(name="hbuf", bufs=2) as hpool,
            tc.tile_pool(name="pbuf", bufs=4) as ppool,
            tc.tile_pool(name="dxbuf", bufs=4) as dpool,
            tc.tile_pool(name="mbuf", bufs=4) as mpool,
            tc.tile_pool(name="d4buf", bufs=4) as d4pool,
            tc.tile_pool(name="mb4buf", bufs=4) as mb4pool,
            tc.tile_pool(name="zbuf", bufs=2) as zpool,
            tc.tile_pool(name="abuf", bufs=2) as apool,
            tc.tile_pool(name="obuf", bufs=2) as opool,
            tc.tile_pool(name="psAB", bufs=2, space="PSUM") as psab,
            tc.tile_pool(name="psU", bufs=2, space="PSUM") as psu,
        ):
            # ---- constants: phase1-critical ones first on sync ------------
            w1_sb = cpool.tile([128, 2, HID], BF16, tag="w1")
            nc.sync.dma_start(out=w1_sb[:], in_=w1d)
            b1_sb = cpool.tile([HID, 1], F32, tag="b1")
            nc.sync.dma_start(out=b1_sb[:], in_=b1c)
            w2_sb = cpool.tile([HID, C], BF16, tag="w2")
            nc.sync.dma_start(out=w2_sb[:], in_=w2d)
            b2_sb = cpool.tile([C, 1], F32, tag="b2")
            nc.sync.dma_start(out=b2_sb[:], in_=b2c)
            W1m_sb = cpool.tile([128, KT, HID], BF16, tag="W1m")
            nc.sync.dma_start(out=W1m_sb[:], in_=W1md)
            V_sb = cpool.tile([C, HID], BF16, tag="V")
            nc.sync.dma_start(out=V_sb[:], in_=Vd)
            bb1_sb = cpool.tile([128, 1], F32, tag="bb1s")
            nc.sync.dma_start(out=bb1_sb[:], in_=bb1s)
            W2_sb = cpool.tile([2 * HID, DOUT], BF16, tag="W2")
            nc.sync.dma_start(out=W2_sb[:], in_=W2d)
            bb2_sb = cpool.tile([DOUT, 1], F32, tag="bb2")
            nc.sync.dma_start(out=bb2_sb[:], in_=bb2c)

            pT = {}
            dxb = {}
            mTr = {}
            dx4 = {}
            mb4 = {}
            ups = {}
            zsb = {}
            aT = {}

            def mm(out_ap, lhsT, mov, start, stop):
                # PE matmul free dim caps at one PSUM bank (512 fp32)
                for h in range(2):
                    hs = slice(h * 512, (h + 1) * 512)
                    nc.tensor.matmul(out_ap[:, hs], lhsT, mov[:, hs],
                                     start=start, stop=stop)

            def phase1(c):
                cs = slice(c * L, (c + 1) * L)
                xt = xpool.tile([128, 2, L], BF16, tag="xt")
                nc.sync.dma_start(out=xt[:], in_=xT[:, :, cs])
                hps = psab.tile([HID, L], F32, tag="psab")
                for k in range(2):
                    mm(hps, w1_sb[:, k, :], xt[:, k, :],
                       start=(k == 0), stop=(k == 1))
                hsb = hpool.tile([HID, L], BF16, tag="hsb")
                nc.scalar.activation(hsb[:], hps[:], RELU,
                                     bias=b1_sb[:], scale=1.0)
                pps = psab.tile([C, L], F32, tag="psab")
                mm(pps, w2_sb[:], hsb[:], start=True, stop=True)
                pT[c] = ppool.tile([C, L], F32, name="pT", tag="pT")
                nc.scalar.add(pT[c][:], pps[:], b2_sb[:])

            def phase2(c):
                # dxb = diff(p) (gpsimd) and m2 = p_t + p_{t-1} (DVE) run in
                # parallel; both bounce through DRAM for the stacked reads.
                p = pT[c]
                dxb[c] = dpool.tile([C, L], BF16, name="dxb", tag="dxb")
                nc.gpsimd.tensor_copy(dxb[c][:, 0:1], p[:, 0:1])
                nc.gpsimd.tensor_sub(dxb[c][:, 1:L], p[:, 1:L], p[:, 0:L - 1])
                mTr[c] = mpool.tile([C, L], BF16, name="mTr", tag="mTr")
                nc.vector.scalar_tensor_tensor(
                    mTr[c][:], dxb[c][:], -0.5, p[:], op0=MUL, op1=ADD,
                )
                # m row (4r+a) -> mTrD[c, a, r, :]: SBUF iterates (r, a, t)
                nc.scalar.dma_start(out=mTrD[c].transpose([1, 0, 2]),
                                    in_=mTr[c][:])
                nc.scalar.dma_start(out=dxbD[c], in_=dxb[c][:])
                # Pair mapping: partition q of tile r <-> (i = 4r + q%4,
                # j = q//4); interleaving keeps source addresses changing
                # between consecutive partitions -> many descriptors -> all
                # 16 DMA engines carry the broadcast.
                # dx4[q] = dx_{q//4}: 1->4 partition broadcast from DRAM
                dx4[c] = d4pool.tile([128, L], BF16, name="dx4", tag="dx4")
                src = dxbD[c].unsqueeze(1).broadcast_to([C, 4, L])
                nc.scalar.dma_start(out=dx4[c][:], in_=src)
                # mb4[q, r, t] = m2_{4r + q%4}[t] = mTrD[c, q%4, r, t]
                mb4[c] = mb4pool.tile([128, KT, L], BF16, name="mb4", tag="mb4")
                for h in range(2):
                    rs = slice(h * (KT // 2), (h + 1) * (KT // 2))
                    src = mTrD[c][:, rs, :].unsqueeze(0)
                    src = src.broadcast_to([32, 4, KT // 2, L])
                    nc.sync.dma_start(out=mb4[c][:, rs, :], in_=src)
                # outer-product rows in place: mb4_r *= dx4  (bf16, DVE 2x)
                for r in range(KT):
                    nc.vector.tensor_mul(mb4[c][:, r, :], mb4[c][:, r, :],
                                         dx4[c][:])

            def mains(c, nparts):
                # batches 0/1 pair-pack one [128, L] PSUM tile; batches 2/3
                # get their own tile (rows 0:64) so the tail stays short
                if nparts == 128 and c % 2 == 0:
                    ups[c // 2] = psu.tile([128, L], F32, name="ups", tag="ups")
                if nparts == 64:
                    upst = psu.tile([128, L], F32, name="ups", tag="ups")
                    ups[c] = upst[0:HID, :]
                    dst = ups[c]
                else:
                    half = c % 2
                    dst = ups[c // 2][64 * half:64 * half + 64, :]
                for r in range(KT):
                    mm(dst, W1m_sb[:, r, :], mb4[c][:, r, :],
                       start=(r == 0), stop=False)
                mm(dst, V_sb[:], dxb[c][:], start=False, stop=True)

            def scan_relu(key, src_c):
                u = ups[key]
                np_ = u.shape[0]
                zs = zpool.tile([128, L], F32, name="zsb", tag="zsb")
                zsb[key] = zs
                nc.vector.tensor_tensor_scan(
                    zs[0:np_, :], u[:], dx4[src_c][0:np_, :], 0.0,
                    op0=ADD, op1=BYP,
                )
                a = apool.tile([128, L], BF16, name="aT", tag="aT")
                aT[key] = a
                nc.scalar.activation(a[0:np_, :], zs[0:np_, :], RELU,
                                     bias=bb1_sb[0:np_, :], scale=1.0)

            def head(c, key, rows):
                cs = slice(c * L, (c + 1) * L)
                ops = psab.tile([DOUT, L], F32, tag="psab")
                mm(ops, W2_sb[rows, :], aT[key][rows, :],
                   start=True, stop=True)
                osb = opool.tile([DOUT, L], F32, tag="osb")
                nc.scalar.add(osb[:], ops[:], bb2_sb[:])
                nc.sync.dma_start(out=out[:, cs], in_=osb[:])

            # ---- software-pipelined emission ------------------------------
            lo, hi = slice(0, 64), slice(64, 128)
            for c in range(nb):
                phase1(c)
                phase2(c)
            mains(0, 128)
            mains(1, 128)
            scan_relu(0, 0)
            mains(2, 64)
            scan_relu(2, 2)
            head(0, 0, lo)
            head(1, 0, hi)
            mains(3, 64)
            scan_relu(3, 3)
            head(2, 2, lo)
            head(3, 3, lo)

    nc.compile()
    return nc


def _w1m_perm(W1):
    # W1m_sb[q, r, :] = 0.5 * W1_sig row (i = 4r + q%4, j = q//4)
    # (the 0.5 compensates m2 = 2*m)
    q = np.arange(128)
    r = np.arange(KT)
    i = 4 * r[None, :] + (q % 4)[:, None]          # [128, KT]
    j = (q // 4)[:, None] + 0 * r[None, :]
    rows = C + 32 * i + j
    return np.ascontiguousarray(
        W1[rows.reshape(-1)].reshape(128, KT, HID))


def host_prep_shared(w1, b1, w2, b2, W1, bb1, W2, bb2):
    import ml_dtypes
    f = np.float32
    bf = ml_dtypes.bfloat16
    return {
        "w1d": np.ascontiguousarray(
            w1.reshape(2, 128, HID).transpose(1, 0, 2)).astype(bf),
        "b1c": np.ascontiguousarray(b1.reshape(-1, 1), f),
        "w2d": np.ascontiguousarray(w2).astype(bf),
        "b2c": np.ascontiguousarray(b2.reshape(-1, 1), f),
        "W1md": _w1m_perm(W1).astype(bf),
        "Vd": np.ascontiguousarray(W1[:C]).astype(bf),
        "bb1s": np.ascontiguousarray(
            np.concatenate([bb1, bb1]).reshape(-1, 1), f),
        "W2d": np.ascontiguousarray(np.vstack([W2, W2])).astype(bf),
        "bb2c": np.ascontiguousarray(bb2.reshape(-1, 1), f),
    }


_NC_CACHE = {}


def _get_nc():
    key = "full"
    if key not in _NC_CACHE:
        _NC_CACHE[key] = build_nc()
    return _NC_CACHE[key]


def kernel(x, w1, b1, w2, b2, W1, bb1, W2, bb2):
    global LAST_EXEC_NS, LAST_PROFILE, LAST_TRACE_PATH
    import ml_dtypes
    nc = _get_nc()
    shared = host_prep_shared(w1, b1, w2, b2, W1, bb1, W2, bb2)
    xbf = np.ascontiguousarray(x, np.float32).astype(ml_dtypes.bfloat16)
    in_maps = []
    for core in range(N_CORES):
        xc = xbf[core * B_CORE:(core + 1) * B_CORE].reshape(T, DIN)
        m = dict(shared)
        # [128, 2, T]: partition q, k-block, time — one DMA per chunk
        m["xT"] = np.ascontiguousarray(
            xc.T.reshape(2, 128, T).transpose(1, 0, 2))
        in_maps.append(m)
    try:
        res = bass_utils.run_bass_kernel_spmd(
            nc, in_maps, core_ids=list(range(N_CORES)), trace=TRACE,
        )
    except Exception:
        if not TRACE:
            raise
        res = bass_utils.run_bass_kernel_spmd(
            nc, in_maps, core_ids=list(range(N_CORES)), trace=False,
        )
    LAST_EXEC_NS = res.exec_time_ns
    LAST_PROFILE = res.profile_json
    LAST_TRACE_PATH = (res.instructions_and_trace or (None, None))[1]
    outs = [np.ascontiguousarray(res.results[i]["out"].T).reshape(B_CORE, L, DOUT)
            for i in range(N_CORES)]
    return np.concatenate(outs, axis=0)
